# revision 20
# baseline (speedup 1.0000x reference)
"""Multi-head attention (B=8, H=8, S=1024, d=128) on 8 TRN2 NeuronCores.

Strategy
--------
- Head-striped sharding: core c processes head c of EVERY batch (8
  slots, one per batch). Per-slot key compaction then uses each batch's
  EXACT tile count (sum ~37 vs 8*max=40 for uniform batch-parallel),
  and tile counts are batch properties so the SPMD program is identical
  across cores.
- Host-side prep (layout only): per (batch b, head c): compact K/V to
  the seq_mask-selected rows (zero-padded to T_b*128), pre-transpose so
  contraction dims land on SBUF partitions, pre-arrange V/ind into the
  SBUF tile layout so every DMA is a plain 2D copy. Q/K in bf16
  (PE-native 1-cycle/column), V/ind in fp8 e4m3.
- All input DMAs are issued up front (everything stays resident in
  SBUF) split across the sync and gpsimd queues, so steady-state
  compute never waits on DMA. Q/K/V/ind all bf16.
- Device math, software-pipelined over a flat (slot, k-tile) sequence
  (QK/exp of tile i+1 issue before the consumers of tile i, crossing
  slot boundaries so the ACT engine never drains):
    logitsT[k, q]  = K^T[:, kt].T @ Q^T     (PE, M=128 bf16, FWL)
    W^T[k, q]      = exp(logitsT * d^-0.5)  (ACT, PSUM -> SBUF bf16)
    den[q]        += ind[kt].T @ W^T        (PE, M=32 pair, accum)
    outT[d, q]    += V[kt].T   @ W^T        (PE, M=128, accum)
  (fp8 weights/V with DoubleRow AV were tried and are ~2x faster on PE
  but the e4m3 3-bit mantissa alone costs 2-4e-2 absmax error -- over
  the tolerance; bf16 it is.)
  then copy outT (fp32->fp16) / den rows to SBUF (DVE) and DMA out; the
  division happens on the host. The learned scalar bias b cancels in
  softmax (shift invariance) and the -1e30 masking is equivalent to
  dropping masked keys, which the compaction does exactly.
- PSUM budget: pl 2 bufs x 2 banks + po 2 banks + pd 1 bank = 7 of 8.
- Host-side unshard: out[b,:,c*128:...] = (outT / den).T per core c,
  plus uniform-average fallback for a fully-masked batch.
"""
from contextlib import ExitStack

import numpy as np

import concourse.bacc as bacc
import concourse.mybir as mybir
import concourse.tile as tile
from concourse.bass_utils import run_bass_kernel_spmd

F32 = mybir.dt.float32
F16 = mybir.dt.float16
BF16 = mybir.dt.bfloat16

B, S, D, H = 8, 1024, 1024, 8
DH = D // H              # 128, head dim = one partition tile
SCALE = float(DH) ** -0.5

_NC_CACHE: dict[tuple, object] = {}

# build options (overridable for profiling experiments)
OPTS: dict = {}


def _build(tiles: tuple, opts: dict | None = None):
    """Build + compile the per-core kernel; tiles[b] = k-tiles of slot b."""
    opts = opts or {}
    nc = bacc.Bacc("TRN2", target_bir_lowering=False, debug=False)

    kts, qts, vcs, inds, outs = [], [], [], [], []
    for b in range(B):
        KP = tiles[b] * 128
        kts.append(nc.dram_tensor(f"k{b}", [DH, KP], BF16, kind="ExternalInput"))
        qts.append(nc.dram_tensor(f"q{b}", [DH, S], BF16, kind="ExternalInput"))
        vcs.append(nc.dram_tensor(f"v{b}", [DH, KP], BF16, kind="ExternalInput"))
        inds.append(nc.dram_tensor(
            f"i{b}", [DH, tiles[b] * 32], BF16, kind="ExternalInput"))
        outs.append(nc.dram_tensor(f"o{b}", [DH, S], F16, kind="ExternalOutput"))
    den_t = nc.dram_tensor("den_t", [B, 2, 512], F32, kind="ExternalOutput")

    seq = [(b, kt) for b in range(B) for kt in range(tiles[b])]

    with tile.TileContext(nc) as tc, ExitStack() as ctx:
        sb_in = ctx.enter_context(tc.tile_pool(name="sb_in", bufs=1))
        sb_w = ctx.enter_context(tc.tile_pool(name="sb_w", bufs=8))
        sb_out = ctx.enter_context(tc.tile_pool(name="sb_out", bufs=4))
        ps_l = ctx.enter_context(tc.tile_pool(name="ps_l", bufs=2, space="PSUM"))
        ps_o = ctx.enter_context(tc.tile_pool(name="ps_o", bufs=1, space="PSUM"))
        ps_d = ctx.enter_context(tc.tile_pool(name="ps_d", bufs=1, space="PSUM"))

        # ---- all input DMAs up front; slot 0 first, queues alternated ----
        kth, qth, vh, ind_sb = {}, {}, {}, {}
        for b in range(B):
            KP = tiles[b] * 128
            kth[b] = sb_in.tile([128, KP], BF16, tag=f"kth{b}",
                                name=f"kth{b}")
            qth[b] = sb_in.tile([128, S], BF16, tag=f"qth{b}",
                                name=f"qth{b}")
            vh[b] = sb_in.tile([128, KP], BF16, tag=f"vh{b}", name=f"vh{b}")
            ind_sb[b] = sb_in.tile([128, tiles[b] * 32], BF16,
                                   tag=f"ind{b}", name=f"ind{b}")
        for b in range(B):
            q1, q2 = (nc.sync, nc.gpsimd) if b % 2 == 0 else (nc.gpsimd, nc.sync)
            q1.dma_start(kth[b][:], kts[b].ap()[:, :])
            q2.dma_start(qth[b][:], qts[b].ap()[:, :])
            q2.dma_start(vh[b][:], vcs[b].ap()[:, :])
            q1.dma_start(ind_sb[b][:], inds[b].ap()[:, :])

        s0, s1 = slice(0, 512), slice(512, 1024)
        wt = {}      # (b, kt) -> [128, S] bf16 weight tile
        po, pd = {}, {}

        def emit_qk(i):
            b, kt = seq[i]
            pl = ps_l.tile([128, S], F32, tag="pl", name=f"pl_{b}_{kt}")
            kk = slice(kt * 128, kt * 128 + 128)
            nc.tensor.matmul(pl[:, s0], kth[b][:, kk], qth[b][:, s0])
            nc.tensor.matmul(pl[:, s1], kth[b][:, kk], qth[b][:, s1])
            wt[(b, kt)] = sb_w.tile([128, S], BF16, tag="wt",
                                    name=f"wt_{b}_{kt}")
            nc.scalar.activation(
                wt[(b, kt)][:], pl[:], mybir.ActivationFunctionType.Exp,
                scale=SCALE,
            )

        def emit_tail(i):
            b, kt = seq[i]
            T = tiles[b]
            first, last = kt == 0, kt == T - 1
            if first:
                po[b] = ps_o.tile([128, S], F32, tag="po", name=f"po_{b}")
                pd[b] = ps_d.tile([64, 512], F32, tag="pd", name=f"pd_{b}")
            w = wt[(b, kt)]
            ic = slice(kt * 32, kt * 32 + 32)
            dd = slice(kt * 128, kt * 128 + 128)
            # den pair for this k-tile: disjoint col groups, one PSUM bank;
            # rows 0-31 accumulate q-chunk 0, rows 32-63 q-chunk 1
            nc.tensor.matmul(pd[b][0:32, :], ind_sb[b][:, ic], w[:, s0],
                             start=first, stop=last)
            nc.tensor.matmul(pd[b][32:64, :], ind_sb[b][:, ic], w[:, s1],
                             start=first, stop=last)
            # AV: M=128 stationary (FWL), two N=512 halves
            nc.tensor.matmul(po[b][:, s0], vh[b][:, dd], w[:, s0],
                             start=first, stop=last)
            nc.tensor.matmul(po[b][:, s1], vh[b][:, dd], w[:, s1],
                             start=first, stop=last)
            if last:
                sq, gq = ((nc.sync, nc.gpsimd) if b % 2 == 0
                          else (nc.gpsimd, nc.sync))
                # denominator rows 0 (q chunk 0) and 32 (q chunk 1)
                dsb = sb_out.tile([33, 512], F32, tag="dsb",
                                  name=f"dsb_{b}")
                nc.vector.tensor_copy(dsb[:], pd[b][0:33, :])
                gq.dma_start(den_t.ap()[b, 0:1, :], dsb[0:1, :])
                gq.dma_start(den_t.ap()[b, 1:2, :], dsb[32:33, :])
                # numerator to SBUF as fp16 in halves (overlap copy +
                # store), divide on host
                osb = sb_out.tile([128, S], F16, tag="osb", name=f"osb_{b}")
                nc.vector.tensor_copy(osb[:, s0], po[b][:, s0])
                sq.dma_start(outs[b].ap()[:, s0], osb[:, s0])
                nc.vector.tensor_copy(osb[:, s1], po[b][:, s1])
                sq.dma_start(outs[b].ap()[:, s1], osb[:, s1])

        emit_qk(0)
        for i in range(len(seq)):
            if i + 1 < len(seq):
                emit_qk(i + 1)
            emit_tail(i)

    nc.compile()
    return nc


def kernel(memory, query, seq_mask, b):
    memory = np.ascontiguousarray(memory, dtype=np.float32)
    query = np.ascontiguousarray(query, dtype=np.float32)
    seq_mask = np.asarray(seq_mask)
    assert memory.shape == (B, S, 2 * D) and query.shape == (B, S, D)

    import ml_dtypes
    bf16 = ml_dtypes.bfloat16

    counts = [int(np.count_nonzero(seq_mask[i])) for i in range(B)]
    tiles = tuple(max(1, (c + 127) // 128) for c in counts)

    key = (tiles, tuple(sorted(OPTS.items())))
    if key not in _NC_CACHE:
        _NC_CACHE[key] = _build(tiles, OPTS)
    nc = _NC_CACHE[key]

    # host-side prep: per (batch slot b); per-core slices are
    # head-striped (core c <-> head c)
    in_maps = [dict() for _ in range(B)]
    for i in range(B):
        idx = np.flatnonzero(seq_mask[i])
        nb = len(idx)
        T = tiles[i]
        kp = T * 128
        # K^T, V compacted and padded: [D, kp], [kp, D]
        ktb = np.zeros((D, kp), dtype=bf16)
        vcb = np.zeros((kp, D), dtype=np.float32)
        if nb:
            ktb[:, :nb] = memory[i, idx, :D].T
            vcb[:nb] = memory[i, idx, D:]
        # V rearranged to SBUF layout [128, kp]: vh[p, kt*128 + j] =
        # v[kt*128 + p, head*128 + j]  (per-head slice applied per core)
        vre = vcb.reshape(T, 128, D).transpose(1, 0, 2)  # [128, T, D]
        # indicator blocks [128, T*32]
        indb = np.zeros((kp,), dtype=bf16)
        indb[:nb] = 1.0
        ind2 = np.repeat(indb.reshape(T, 128).T[:, :, None], 32, axis=2
                         ).reshape(128, T * 32)
        qtb = query[i].T.astype(bf16)                    # [D, S]
        for c in range(B):
            hs = c * DH
            in_maps[c][f"k{i}"] = ktb[hs:hs + DH]
            in_maps[c][f"q{i}"] = qtb[hs:hs + DH]
            in_maps[c][f"v{i}"] = np.ascontiguousarray(
                vre[:, :, hs:hs + DH].reshape(128, kp)).astype(bf16)
            in_maps[c][f"i{i}"] = ind2

    res = run_bass_kernel_spmd(nc, in_maps, list(range(B)))
    out = np.empty((B, S, D), dtype=np.float32)
    for i in range(B):
        for c in range(B):
            num = res.results[c][f"o{i}"].astype(np.float32)   # [DH, S]
            dd = res.results[c]["den_t"][i]                    # [2, 512]
            den = np.concatenate([dd[0], dd[1]])               # [S]
            with np.errstate(divide="ignore", invalid="ignore"):
                out[i, :, c * DH:(c + 1) * DH] = (num / den[None, :]).T
        if counts[i] == 0:
            # all keys masked: reference softmax degenerates to uniform
            out[i] = memory[i, :, D:].mean(axis=0)[None, :]
    return out


# revision 23
# speedup vs baseline: 1.2788x; 1.2788x over previous
"""Multi-head attention (B=8, H=8, S=1024, d=128) on 8 TRN2 NeuronCores.

Strategy
--------
- Head-striped sharding: core c processes head c of EVERY batch (8
  slots, one per batch). Per-slot key compaction then uses each batch's
  EXACT tile count (sum ~37 vs 8*max=40 for uniform batch-parallel),
  and tile counts are batch properties so the SPMD program is identical
  across cores.
- Host-side prep (layout only): per (batch b, head c): compact K/V to
  the seq_mask-selected rows (zero-padded to T_b*128), pre-transpose so
  contraction dims land on SBUF partitions, pre-arrange V/ind into the
  SBUF tile layout so every DMA is a plain 2D copy. Q/K in bf16
  (PE-native 1-cycle/column), V/ind in fp8 e4m3.
- All input DMAs are issued up front (everything stays resident in
  SBUF) split across the sync and gpsimd queues, so steady-state
  compute never waits on DMA. Q/K/V/ind all bf16.
- Device math, software-pipelined over a flat (slot, k-tile) sequence
  (QK/exp of tile i+1 issue before the consumers of tile i, crossing
  slot boundaries so the ACT engine never drains):
    logitsT[k, q]  = K^T[:, kt].T @ Q^T     (PE, M=128 bf16, FWL)
    W^T[k, q]      = exp(logitsT * d^-0.5)  (ACT, PSUM -> SBUF bf16)
    den[q]        += ind[kt].T @ W^T        (PE, M=32 pair, accum)
    outT[d, q]    += V[kt].T   @ W^T        (PE, M=128, accum)
  (fp8 weights/V with DoubleRow AV were tried and are ~2x faster on PE
  but the e4m3 3-bit mantissa alone costs 2-4e-2 absmax error -- over
  the tolerance; bf16 it is.)
  then copy outT (fp32->fp16) / den rows to SBUF (DVE) and DMA out; the
  division happens on the host. The learned scalar bias b cancels in
  softmax (shift invariance) and the -1e30 masking is equivalent to
  dropping masked keys, which the compaction does exactly.
- PSUM budget: pl 2 bufs x 2 banks + po 2 banks + pd 1 bank = 7 of 8.
- Host-side unshard: out[b,:,c*128:...] = (outT / den).T per core c,
  plus uniform-average fallback for a fully-masked batch.
"""
from contextlib import ExitStack

import numpy as np

import concourse.bacc as bacc
import concourse.mybir as mybir
import concourse.tile as tile
from concourse.bass_utils import run_bass_kernel_spmd

F32 = mybir.dt.float32
F16 = mybir.dt.float16
BF16 = mybir.dt.bfloat16

B, S, D, H = 8, 1024, 1024, 8
DH = D // H              # 128, head dim = one partition tile
SCALE = float(DH) ** -0.5

_NC_CACHE: dict[tuple, object] = {}

# build options (overridable for profiling experiments)
OPTS: dict = {}


def _build(tiles: tuple, opts: dict | None = None):
    """Build + compile the per-core kernel; tiles[b] = k-tiles of slot b."""
    opts = opts or {}
    nc = bacc.Bacc("TRN2", target_bir_lowering=False, debug=False)

    kts, qts, vcs, inds, outs = [], [], [], [], []
    for b in range(B):
        KP = tiles[b] * 128
        kts.append(nc.dram_tensor(f"k{b}", [DH, KP], BF16, kind="ExternalInput"))
        qts.append(nc.dram_tensor(f"q{b}", [DH, S], BF16, kind="ExternalInput"))
        vcs.append(nc.dram_tensor(f"v{b}", [DH, KP], BF16, kind="ExternalInput"))
        inds.append(nc.dram_tensor(
            f"i{b}", [DH, tiles[b] * 32], BF16, kind="ExternalInput"))
        outs.append(nc.dram_tensor(f"o{b}", [DH, S], F16, kind="ExternalOutput"))
    den_t = nc.dram_tensor("den_t", [B, 2, 512], F32, kind="ExternalOutput")

    seq = [(b, kt) for b in range(B) for kt in range(tiles[b])]

    with tile.TileContext(nc) as tc, ExitStack() as ctx:
        sb_in = ctx.enter_context(tc.tile_pool(name="sb_in", bufs=1))
        sb_w = ctx.enter_context(tc.tile_pool(name="sb_w", bufs=8))
        sb_out = ctx.enter_context(tc.tile_pool(name="sb_out", bufs=4))
        ps_l = ctx.enter_context(tc.tile_pool(name="ps_l", bufs=2, space="PSUM"))
        ps_o = ctx.enter_context(tc.tile_pool(name="ps_o", bufs=1, space="PSUM"))
        ps_d = ctx.enter_context(tc.tile_pool(name="ps_d", bufs=1, space="PSUM"))

        # ---- all input DMAs up front; slot 0 first, queues alternated.
        # slot 0's first k-tile / q-chunk get their own small tiles so the
        # very first matmul releases as soon as ~160KB has landed (tile-
        # granular dependency tracking would otherwise gate it on the full
        # load).
        kth, qth, vh, ind_sb = {}, {}, {}, {}
        k0a = sb_in.tile([128, 128], BF16, tag="k0a", name="k0a")
        q0a = sb_in.tile([128, 512], BF16, tag="q0a", name="q0a")
        for b in range(B):
            KP = tiles[b] * 128
            kth[b] = sb_in.tile([128, KP], BF16, tag=f"kth{b}",
                                name=f"kth{b}")
            qth[b] = sb_in.tile([128, S], BF16, tag=f"qth{b}",
                                name=f"qth{b}")
            vh[b] = sb_in.tile([128, KP], BF16, tag=f"vh{b}", name=f"vh{b}")
            ind_sb[b] = sb_in.tile([128, tiles[b] * 32], BF16,
                                   tag=f"ind{b}", name=f"ind{b}")
        nc.sync.dma_start(k0a[:], kts[0].ap()[:, 0:128])
        nc.gpsimd.dma_start(q0a[:], qts[0].ap()[:, 0:512])
        for b in range(B):
            q1, q2 = (nc.sync, nc.gpsimd) if b % 2 == 0 else (nc.gpsimd, nc.sync)
            q1.dma_start(kth[b][:], kts[b].ap()[:, :])
            q2.dma_start(qth[b][:], qts[b].ap()[:, :])
            q2.dma_start(vh[b][:], vcs[b].ap()[:, :])
            q1.dma_start(ind_sb[b][:], inds[b].ap()[:, :])

        s0, s1 = slice(0, 512), slice(512, 1024)
        wt = {}      # (b, kt) -> [128, S] bf16 weight tile
        po, pd = {}, {}

        def emit_qk(i):
            b, kt = seq[i]
            pl = ps_l.tile([128, S], F32, tag="pl", name=f"pl_{b}_{kt}")
            kk = slice(kt * 128, kt * 128 + 128)
            if i == 0:
                # fast-start tiles: only ~160KB of DMA gates this matmul
                nc.tensor.matmul(pl[:, s0], k0a[:], q0a[:])
                nc.tensor.matmul(pl[:, s1], k0a[:], qth[b][:, s1])
            else:
                nc.tensor.matmul(pl[:, s0], kth[b][:, kk], qth[b][:, s0])
                nc.tensor.matmul(pl[:, s1], kth[b][:, kk], qth[b][:, s1])
            wt[(b, kt)] = sb_w.tile([128, S], BF16, tag="wt",
                                    name=f"wt_{b}_{kt}")
            nc.scalar.activation(
                wt[(b, kt)][:], pl[:], mybir.ActivationFunctionType.Exp,
                scale=SCALE,
            )

        def emit_tail(i):
            b, kt = seq[i]
            T = tiles[b]
            first, last = kt == 0, kt == T - 1
            if first:
                po[b] = ps_o.tile([128, S], F32, tag="po", name=f"po_{b}")
                pd[b] = ps_d.tile([64, 512], F32, tag="pd", name=f"pd_{b}")
            w = wt[(b, kt)]
            ic = slice(kt * 32, kt * 32 + 32)
            dd = slice(kt * 128, kt * 128 + 128)
            # den pair for this k-tile: disjoint col groups, one PSUM bank;
            # rows 0-31 accumulate q-chunk 0, rows 32-63 q-chunk 1
            nc.tensor.matmul(pd[b][0:32, :], ind_sb[b][:, ic], w[:, s0],
                             start=first, stop=last)
            nc.tensor.matmul(pd[b][32:64, :], ind_sb[b][:, ic], w[:, s1],
                             start=first, stop=last)
            # AV: M=128 stationary (FWL), two N=512 halves
            nc.tensor.matmul(po[b][:, s0], vh[b][:, dd], w[:, s0],
                             start=first, stop=last)
            nc.tensor.matmul(po[b][:, s1], vh[b][:, dd], w[:, s1],
                             start=first, stop=last)
            if last:
                sq, gq = ((nc.sync, nc.gpsimd) if b % 2 == 0
                          else (nc.gpsimd, nc.sync))
                # denominator rows 0 (q chunk 0) and 32 (q chunk 1)
                dsb = sb_out.tile([33, 512], F32, tag="dsb",
                                  name=f"dsb_{b}")
                nc.vector.tensor_copy(dsb[:], pd[b][0:33, :])
                gq.dma_start(den_t.ap()[b, 0:1, :], dsb[0:1, :])
                gq.dma_start(den_t.ap()[b, 1:2, :], dsb[32:33, :])
                # numerator to SBUF as fp16 in halves (overlap copy +
                # store), divide on host
                osb = sb_out.tile([128, S], F16, tag="osb", name=f"osb_{b}")
                nc.vector.tensor_copy(osb[:, s0], po[b][:, s0])
                sq.dma_start(outs[b].ap()[:, s0], osb[:, s0])
                nc.vector.tensor_copy(osb[:, s1], po[b][:, s1])
                gq.dma_start(outs[b].ap()[:, s1], osb[:, s1])

        # two-deep lookahead: QK(i+2) is gated on the same exp(i)
        # completion as den/AV(i) and issues first, so it fills the PE
        # while exp(i+1) runs and ACT stays saturated
        n = len(seq)
        emit_qk(0)
        if n > 1:
            emit_qk(1)
        for i in range(n):
            if i + 2 < n:
                emit_qk(i + 2)
            emit_tail(i)

    nc.compile()
    return nc


def kernel(memory, query, seq_mask, b):
    memory = np.ascontiguousarray(memory, dtype=np.float32)
    query = np.ascontiguousarray(query, dtype=np.float32)
    seq_mask = np.asarray(seq_mask)
    assert memory.shape == (B, S, 2 * D) and query.shape == (B, S, D)

    import ml_dtypes
    bf16 = ml_dtypes.bfloat16

    counts = [int(np.count_nonzero(seq_mask[i])) for i in range(B)]
    tiles = tuple(max(1, (c + 127) // 128) for c in counts)

    key = (tiles, tuple(sorted(OPTS.items())))
    if key not in _NC_CACHE:
        _NC_CACHE[key] = _build(tiles, OPTS)
    nc = _NC_CACHE[key]

    # host-side prep: per (batch slot b); per-core slices are
    # head-striped (core c <-> head c)
    in_maps = [dict() for _ in range(B)]
    for i in range(B):
        idx = np.flatnonzero(seq_mask[i])
        nb = len(idx)
        T = tiles[i]
        kp = T * 128
        # K^T, V compacted and padded: [D, kp], [kp, D]
        ktb = np.zeros((D, kp), dtype=bf16)
        vcb = np.zeros((kp, D), dtype=np.float32)
        if nb:
            ktb[:, :nb] = memory[i, idx, :D].T
            vcb[:nb] = memory[i, idx, D:]
        # V rearranged to SBUF layout [128, kp]: vh[p, kt*128 + j] =
        # v[kt*128 + p, head*128 + j]  (per-head slice applied per core)
        vre = vcb.reshape(T, 128, D).transpose(1, 0, 2)  # [128, T, D]
        # indicator blocks [128, T*32]
        indb = np.zeros((kp,), dtype=bf16)
        indb[:nb] = 1.0
        ind2 = np.repeat(indb.reshape(T, 128).T[:, :, None], 32, axis=2
                         ).reshape(128, T * 32)
        qtb = query[i].T.astype(bf16)                    # [D, S]
        for c in range(B):
            hs = c * DH
            in_maps[c][f"k{i}"] = ktb[hs:hs + DH]
            in_maps[c][f"q{i}"] = qtb[hs:hs + DH]
            in_maps[c][f"v{i}"] = np.ascontiguousarray(
                vre[:, :, hs:hs + DH].reshape(128, kp)).astype(bf16)
            in_maps[c][f"i{i}"] = ind2

    res = run_bass_kernel_spmd(nc, in_maps, list(range(B)))
    out = np.empty((B, S, D), dtype=np.float32)
    for i in range(B):
        for c in range(B):
            num = res.results[c][f"o{i}"].astype(np.float32)   # [DH, S]
            dd = res.results[c]["den_t"][i]                    # [2, 512]
            den = np.concatenate([dd[0], dd[1]])               # [S]
            with np.errstate(divide="ignore", invalid="ignore"):
                out[i, :, c * DH:(c + 1) * DH] = (num / den[None, :]).T
        if counts[i] == 0:
            # all keys masked: reference softmax degenerates to uniform
            out[i] = memory[i, :, D:].mean(axis=0)[None, :]
    return out


# revision 30
# speedup vs baseline: 1.2923x; 1.0106x over previous
"""Multi-head attention (B=8, H=8, S=1024, d=128) on 8 TRN2 NeuronCores.

Strategy
--------
- Head-striped sharding: core c processes head c of EVERY batch (8
  slots, one per batch). Per-slot key compaction then uses each batch's
  EXACT tile count (sum ~37 vs 8*max=40 for uniform batch-parallel),
  and tile counts are batch properties so the SPMD program is identical
  across cores.
- Host-side prep (layout only): per (batch b, head c): compact K/V to
  the seq_mask-selected rows (zero-padded to T_b*128), pre-transpose so
  contraction dims land on SBUF partitions, pre-arrange V/ind into the
  SBUF tile layout so every DMA is a plain 2D copy. Q/K in bf16
  (PE-native 1-cycle/column), V/ind in fp8 e4m3.
- All input DMAs are issued up front (everything stays resident in
  SBUF) split across the sync and gpsimd queues, so steady-state
  compute never waits on DMA. Q/K/V/ind all bf16.
- Device math, software-pipelined over a flat (slot, k-tile) sequence
  (QK/exp of tile i+1 issue before the consumers of tile i, crossing
  slot boundaries so the ACT engine never drains):
    logitsT[k, q]  = K^T[:, kt].T @ Q^T     (PE, M=128 bf16, FWL)
    W^T[k, q]      = exp(logitsT * d^-0.5)  (ACT, PSUM -> SBUF bf16)
    den[q]        += ind[kt].T @ W^T        (PE, M=32 pair, accum)
    outT[d, q]    += V[kt].T   @ W^T        (PE, M=128, accum)
  (fp8 weights/V with DoubleRow AV were tried and are ~2x faster on PE
  but the e4m3 3-bit mantissa alone costs 2-4e-2 absmax error -- over
  the tolerance; bf16 it is.)
  then copy outT (fp32->fp16) / den rows to SBUF (DVE) and DMA out; the
  division happens on the host. The learned scalar bias b cancels in
  softmax (shift invariance) and the -1e30 masking is equivalent to
  dropping masked keys, which the compaction does exactly.
- PSUM budget: pl 2 bufs x 2 banks + po 2 banks + pd 1 bank = 7 of 8.
- Host-side unshard: out[b,:,c*128:...] = (outT / den).T per core c,
  plus uniform-average fallback for a fully-masked batch.
"""
from contextlib import ExitStack

import numpy as np

import concourse.bacc as bacc
import concourse.mybir as mybir
import concourse.tile as tile
from concourse.bass_utils import run_bass_kernel_spmd

F32 = mybir.dt.float32
F16 = mybir.dt.float16
BF16 = mybir.dt.bfloat16

B, S, D, H = 8, 1024, 1024, 8
DH = D // H              # 128, head dim = one partition tile
SCALE = float(DH) ** -0.5

_NC_CACHE: dict[tuple, object] = {}

# build options (overridable for profiling experiments)
OPTS: dict = {}


def _build(tiles: tuple, opts: dict | None = None):
    """Build + compile the per-core kernel; tiles[b] = k-tiles of slot b."""
    opts = opts or {}
    nc = bacc.Bacc("TRN2", target_bir_lowering=False, debug=False)

    kts, qts, vcs, inds, outs = [], [], [], [], []
    for b in range(B):
        KP = tiles[b] * 128
        kts.append(nc.dram_tensor(f"k{b}", [DH, KP], BF16, kind="ExternalInput"))
        qts.append(nc.dram_tensor(f"q{b}", [DH, S], BF16, kind="ExternalInput"))
        vcs.append(nc.dram_tensor(f"v{b}", [DH, KP], BF16, kind="ExternalInput"))
        inds.append(nc.dram_tensor(
            f"i{b}", [DH, tiles[b] * 32], BF16, kind="ExternalInput"))
        outs.append(nc.dram_tensor(f"o{b}", [DH, S], F16, kind="ExternalOutput"))
    den_t = nc.dram_tensor("den_t", [B, 2, 512], F32, kind="ExternalOutput")

    seq = [(b, kt) for b in range(B) for kt in range(tiles[b])]

    with tile.TileContext(nc) as tc, ExitStack() as ctx:
        sb_in = ctx.enter_context(tc.tile_pool(name="sb_in", bufs=1))
        sb_w = ctx.enter_context(tc.tile_pool(name="sb_w", bufs=8))
        sb_out = ctx.enter_context(tc.tile_pool(name="sb_out", bufs=4))
        ps_l = ctx.enter_context(tc.tile_pool(name="ps_l", bufs=2, space="PSUM"))
        # po split by q-chunk: s0 half double-buffered so the next slot's
        # AV does not stall on this slot's output CASTs
        ps_o0 = ctx.enter_context(tc.tile_pool(name="ps_o0", bufs=2, space="PSUM"))
        ps_o1 = ctx.enter_context(tc.tile_pool(name="ps_o1", bufs=1, space="PSUM"))
        ps_d = ctx.enter_context(tc.tile_pool(name="ps_d", bufs=1, space="PSUM"))

        # ---- all input DMAs up front; slot 0 first, queues alternated.
        # slot 0's first k-tile / q-chunk get their own small tiles so the
        # very first matmul releases as soon as ~160KB has landed (tile-
        # granular dependency tracking would otherwise gate it on the full
        # load).
        kth, qth, vh, ind_sb = {}, {}, {}, {}
        k0a = sb_in.tile([128, 128], BF16, tag="k0a", name="k0a")
        q0a = sb_in.tile([128, 512], BF16, tag="q0a", name="q0a")
        for b in range(B):
            KP = tiles[b] * 128
            kth[b] = sb_in.tile([128, KP], BF16, tag=f"kth{b}",
                                name=f"kth{b}")
            qth[b] = sb_in.tile([128, S], BF16, tag=f"qth{b}",
                                name=f"qth{b}")
            vh[b] = sb_in.tile([128, KP], BF16, tag=f"vh{b}", name=f"vh{b}")
            ind_sb[b] = sb_in.tile([128, tiles[b] * 32], BF16,
                                   tag=f"ind{b}", name=f"ind{b}")
        nc.sync.dma_start(k0a[:], kts[0].ap()[:, 0:128])
        nc.gpsimd.dma_start(q0a[:], qts[0].ap()[:, 0:512])

        # PE warm-up: ~4us of dummy matmuls on a memset tile, issued while
        # the first input DMAs are still in flight, so the HAM clock gate
        # reaches 8/8 before real work starts. Output goes to a pl-pool
        # buffer that recycles afterwards; nothing reads it.
        warm = sb_in.tile([128, 64], BF16, tag="warm", name="warm")
        nc.vector.memset(warm[:], 0.0)
        wpl = ps_l.tile([128, S], F32, tag="pl", name="pl_warm")
        for _ in range(48):
            nc.tensor.matmul(wpl[0:16, 0:64], warm[:, 0:16], warm[:, 0:64])
        for b in range(B):
            q1, q2 = (nc.sync, nc.gpsimd) if b % 2 == 0 else (nc.gpsimd, nc.sync)
            q1.dma_start(kth[b][:], kts[b].ap()[:, :])
            q2.dma_start(qth[b][:], qts[b].ap()[:, :])
            q2.dma_start(vh[b][:], vcs[b].ap()[:, :])
            q1.dma_start(ind_sb[b][:], inds[b].ap()[:, :])

        s0, s1 = slice(0, 512), slice(512, 1024)
        wt = {}      # (b, kt) -> [128, S] bf16 weight tile
        po, pd = {}, {}

        def emit_qk(i):
            b, kt = seq[i]
            pl = ps_l.tile([128, S], F32, tag="pl", name=f"pl_{b}_{kt}")
            kk = slice(kt * 128, kt * 128 + 128)
            if i == 0:
                # fast-start tiles: only ~160KB of DMA gates this matmul
                nc.tensor.matmul(pl[:, s0], k0a[:], q0a[:])
                nc.tensor.matmul(pl[:, s1], k0a[:], qth[b][:, s1])
            else:
                nc.tensor.matmul(pl[:, s0], kth[b][:, kk], qth[b][:, s0])
                nc.tensor.matmul(pl[:, s1], kth[b][:, kk], qth[b][:, s1])
            wt[(b, kt)] = sb_w.tile([128, S], BF16, tag="wt",
                                    name=f"wt_{b}_{kt}")
            nc.scalar.activation(
                wt[(b, kt)][:], pl[:], mybir.ActivationFunctionType.Exp,
                scale=SCALE,
            )

        def emit_tail(i):
            b, kt = seq[i]
            T = tiles[b]
            first, last = kt == 0, kt == T - 1
            if first:
                po[b] = (ps_o0.tile([128, 512], F32, tag="po0",
                                    name=f"po0_{b}"),
                         ps_o1.tile([128, 512], F32, tag="po1",
                                    name=f"po1_{b}"))
                pd[b] = ps_d.tile([64, 512], F32, tag="pd", name=f"pd_{b}")
            w = wt[(b, kt)]
            ic = slice(kt * 32, kt * 32 + 32)
            dd = slice(kt * 128, kt * 128 + 128)
            # den pair for this k-tile: disjoint col groups, one PSUM bank;
            # rows 0-31 accumulate q-chunk 0, rows 32-63 q-chunk 1
            nc.tensor.matmul(pd[b][0:32, :], ind_sb[b][:, ic], w[:, s0],
                             start=first, stop=last)
            nc.tensor.matmul(pd[b][32:64, :], ind_sb[b][:, ic], w[:, s1],
                             start=first, stop=last)
            # AV: M=128 stationary (FWL), two N=512 halves
            nc.tensor.matmul(po[b][0][:], vh[b][:, dd], w[:, s0],
                             start=first, stop=last)
            nc.tensor.matmul(po[b][1][:], vh[b][:, dd], w[:, s1],
                             start=first, stop=last)
            if last:
                sq, gq = ((nc.sync, nc.gpsimd) if b % 2 == 0
                          else (nc.gpsimd, nc.sync))
                # denominator rows 0 (q chunk 0) and 32 (q chunk 1)
                dsb = sb_out.tile([33, 512], F32, tag="dsb",
                                  name=f"dsb_{b}")
                nc.vector.tensor_copy(dsb[:], pd[b][0:33, :])
                gq.dma_start(den_t.ap()[b, 0:1, :], dsb[0:1, :])
                gq.dma_start(den_t.ap()[b, 1:2, :], dsb[32:33, :])
                # numerator to SBUF as fp16 in halves (overlap copy +
                # store), divide on host
                osb = sb_out.tile([128, S], F16, tag="osb", name=f"osb_{b}")
                nc.vector.tensor_copy(osb[:, s0], po[b][0][:])
                sq.dma_start(outs[b].ap()[:, s0], osb[:, s0])
                if i == len(seq) - 1:
                    # final slot: second half via the (now idle) scalar
                    # engine so both copies run concurrently at the tail
                    nc.scalar.copy(osb[:, s1], po[b][1][:])
                else:
                    nc.vector.tensor_copy(osb[:, s1], po[b][1][:])
                gq.dma_start(outs[b].ap()[:, s1], osb[:, s1])

        # two-deep lookahead: QK(i+2) is gated on the same exp(i)
        # completion as den/AV(i) and issues first, so it fills the PE
        # while exp(i+1) runs and ACT stays saturated
        n = len(seq)
        emit_qk(0)
        if n > 1:
            emit_qk(1)
        for i in range(n):
            if i + 2 < n:
                emit_qk(i + 2)
            emit_tail(i)

    nc.compile()
    return nc


def kernel(memory, query, seq_mask, b):
    memory = np.ascontiguousarray(memory, dtype=np.float32)
    query = np.ascontiguousarray(query, dtype=np.float32)
    seq_mask = np.asarray(seq_mask)
    assert memory.shape == (B, S, 2 * D) and query.shape == (B, S, D)

    import ml_dtypes
    bf16 = ml_dtypes.bfloat16

    counts = [int(np.count_nonzero(seq_mask[i])) for i in range(B)]
    btiles = [max(1, (c + 127) // 128) for c in counts]
    # slots ordered by descending tile count: canonical compile key and
    # a short final slot at the kernel tail
    order = sorted(range(B), key=lambda i: -btiles[i])
    slot_of = {batch: j for j, batch in enumerate(order)}
    tiles = tuple(btiles[i] for i in order)

    key = (tiles, tuple(sorted(OPTS.items())))
    if key not in _NC_CACHE:
        _NC_CACHE[key] = _build(tiles, OPTS)
    nc = _NC_CACHE[key]

    # host-side prep: per (batch slot b); per-core slices are
    # head-striped (core c <-> head c)
    in_maps = [dict() for _ in range(B)]
    for i in range(B):
        sl = slot_of[i]
        idx = np.flatnonzero(seq_mask[i])
        nb = len(idx)
        T = btiles[i]
        kp = T * 128
        # K^T, V compacted and padded: [D, kp], [kp, D]
        ktb = np.zeros((D, kp), dtype=bf16)
        vcb = np.zeros((kp, D), dtype=np.float32)
        if nb:
            ktb[:, :nb] = memory[i, idx, :D].T
            vcb[:nb] = memory[i, idx, D:]
        # V rearranged to SBUF layout [128, kp]: vh[p, kt*128 + j] =
        # v[kt*128 + p, head*128 + j]  (per-head slice applied per core)
        vre = vcb.reshape(T, 128, D).transpose(1, 0, 2)  # [128, T, D]
        # indicator blocks [128, T*32]
        indb = np.zeros((kp,), dtype=bf16)
        indb[:nb] = 1.0
        ind2 = np.repeat(indb.reshape(T, 128).T[:, :, None], 32, axis=2
                         ).reshape(128, T * 32)
        qtb = query[i].T.astype(bf16)                    # [D, S]
        for c in range(B):
            hs = c * DH
            in_maps[c][f"k{sl}"] = ktb[hs:hs + DH]
            in_maps[c][f"q{sl}"] = qtb[hs:hs + DH]
            in_maps[c][f"v{sl}"] = np.ascontiguousarray(
                vre[:, :, hs:hs + DH].reshape(128, kp)).astype(bf16)
            in_maps[c][f"i{sl}"] = ind2

    res = run_bass_kernel_spmd(nc, in_maps, list(range(B)))
    out = np.empty((B, S, D), dtype=np.float32)
    for i in range(B):
        sl = slot_of[i]
        for c in range(B):
            num = res.results[c][f"o{sl}"].astype(np.float32)  # [DH, S]
            dd = res.results[c]["den_t"][sl]                   # [2, 512]
            den = np.concatenate([dd[0], dd[1]])               # [S]
            with np.errstate(divide="ignore", invalid="ignore"):
                out[i, :, c * DH:(c + 1) * DH] = (num / den[None, :]).T
        if counts[i] == 0:
            # all keys masked: reference softmax degenerates to uniform
            out[i] = memory[i, :, D:].mean(axis=0)[None, :]
    return out
